# revision 1
# baseline (speedup 1.0000x reference)
"""GraphSAGE (3-layer) Trainium2 Bass kernel, 8-core SPMD.

Strategy (graph/data parallel, per sharding hint):
  - Nodes padded to 50176 = 8*6272; core c owns dst nodes [c*6272, (c+1)*6272).
  - Edges bucketed by (dst tile of 128 nodes); mean-aggregation done as PE
    matmuls: for each chunk of 128 edges, psum += onehotT.T @ msgs, where
    msgs = dma_gather(h_table[src]) and onehotT built on the Vector engine as
    is_equal(iota_row_matrix, dstloc_per_edge) (pad slots hold 128 -> zero
    column -> no contribution). Gathers are SWDGE-descriptor-rate bound, so
    keeping the one-hot off the gather path halves GPSIMD time.
  - dma_gather indices are int16 (<=32767), so each tile's edges are split in
    a "lo" group (src < 25088, gathered from table base 0) and a "hi" group
    (src >= 25088, gathered from table base 17408 with idx = src - 17408).
  - Linear: out^T = W_self^T @ h_self^T + W_neigh^T @ h_neigh^T on PE
    (transposes via PE identity matmul), bias+ReLU on ACT, then transpose
    back to row-major for the next layer's gather table.
  - Inter-layer: each core's block is AllGather'ed (HBM collective) into a
    full replicated bf16 table for the next layer's gathers.
"""

import sys

if "/opt/trn_rl_repo" not in sys.path:
    sys.path.insert(0, "/opt/trn_rl_repo")

from contextlib import ExitStack

import numpy as np
import ml_dtypes

N_NODES = 50000
F = 128
OUT_F = 64
NCORES = 8
NLOC = 6272          # nodes per core
NTILES = 49          # 6272 / 128
NPAD = NCORES * NLOC  # 50176
P = 128
SPLIT = 25088        # lo: src < SPLIT ; hi: src >= SPLIT
HI_BASE = 17408      # hi gather base; idx = src - HI_BASE  (max 50175-17408=32767)
IDROWS = 192         # identity gather table rows (128 identity + zero rows)

_prog_cache = {}


def _wrap_idx(a):
    """[T, n] idx stream -> dma_gather wrapped layout [128, T, n/16] int16.

    wrapped[p, t, s] = a[t, s*16 + p%16]  (replicated across the 8 Q7 cores).
    """
    T, n = a.shape
    w = a.reshape(T, n // 16, 16).transpose(2, 0, 1)      # [16, T, n/16]
    w = np.tile(w, (8, 1, 1))                              # [128, T, n/16]
    return np.ascontiguousarray(w.astype(np.int16))


def _preprocess(src, dst):
    """Bucket edges by (core,tile), split lo/hi by src, pad to uniform chunks."""
    src = src.astype(np.int64)
    dst = dst.astype(np.int64)
    E = src.shape[0]

    gtile = dst // P            # global tile id 0..391 (dst tile of 128 nodes)
    dstloc = dst % P
    lo = src < SPLIT

    key = gtile * 2 + (~lo).astype(np.int64)   # lo group first within tile
    order = np.argsort(key, kind="stable")
    counts = np.bincount(key, minlength=NCORES * NTILES * 2)
    lo_counts = counts[0::2].reshape(NCORES, NTILES)
    hi_counts = counts[1::2].reshape(NCORES, NTILES)

    NLO = int(np.ceil(lo_counts.max() / P))
    NHI = int(np.ceil(hi_counts.max() / P))
    NCH = NLO + NHI

    # slot arrays per global tile
    src_slot = np.zeros((NCORES * NTILES, NCH * P), np.int64)
    oh_slot = np.full((NCORES * NTILES, NCH * P), P, np.int64)  # 128 -> zero row

    skey = key[order]
    group_start = np.zeros(NCORES * NTILES * 2 + 1, np.int64)
    np.cumsum(counts, out=group_start[1:])
    pos_in_group = np.arange(E) - group_start[skey]
    row = gtile[order]
    grp = skey % 2
    col = pos_in_group + grp * (NLO * P)
    sv = src[order]
    src_slot[row, col] = np.where(grp == 0, sv, sv - HI_BASE)
    oh_slot[row, col] = dstloc[order]

    deg = np.bincount(dst, minlength=NPAD).astype(np.float32)
    inv_deg = 1.0 / np.maximum(deg, 1.0)

    per_core = []
    for c in range(NCORES):
        sl = src_slot[c * NTILES:(c + 1) * NTILES]
        ol = oh_slot[c * NTILES:(c + 1) * NTILES]
        idxlo = _wrap_idx(sl[:, : NLO * P])
        idxhi = _wrap_idx(sl[:, NLO * P:])
        # dstloc values, edge-partitioned: [128, NTILES, NCH] bf16
        # (pad slots hold 128 -> never equal to iota 0..127 -> zero column)
        import ml_dtypes as _md
        dstlocf = np.ascontiguousarray(
            ol.reshape(NTILES, NCH_ := ol.shape[1] // P, P).transpose(2, 0, 1)
        ).astype(np.float32)
        invd = inv_deg[c * NLOC:(c + 1) * NLOC].reshape(NTILES, P).T.copy()  # [128, 49]
        per_core.append(dict(idxlo=idxlo, idxhi=idxhi, dstlocf=dstlocf, invdeg=invd))
    return per_core, NLO, NHI


def _build_program(NLO, NHI):
    import concourse.bacc as bacc
    import concourse.bass as bass
    import concourse.mybir as mybir
    import concourse.tile as tile

    dt = mybir.dt
    NCH = NLO + NHI
    nc = bacc.Bacc("TRN2", target_bir_lowering=False, debug=False,
                   num_devices=NCORES, dynamic_dma_scratch_size=49152)

    htab0 = nc.dram_tensor("htab0", [NPAD, F], dt.bfloat16, kind="ExternalInput")
    hself0 = nc.dram_tensor("hself0", [P, NTILES, F], dt.bfloat16, kind="ExternalInput")
    idxlo = nc.dram_tensor("idxlo", [P, NTILES, NLO * 8], dt.int16, kind="ExternalInput")
    idxhi = nc.dram_tensor("idxhi", [P, NTILES, NHI * 8], dt.int16, kind="ExternalInput")
    dstlocf = nc.dram_tensor("dstlocf", [P, NTILES, NCH], dt.float32, kind="ExternalInput")
    iotam = nc.dram_tensor("iotam", [P, P], dt.float32, kind="ExternalInput")
    invdeg = nc.dram_tensor("invdeg", [P, NTILES], dt.float32, kind="ExternalInput")
    ident = nc.dram_tensor("ident", [P, P], dt.bfloat16, kind="ExternalInput")
    identf = nc.dram_tensor("identf", [OUT_F, OUT_F], dt.float32, kind="ExternalInput")
    ws = [nc.dram_tensor(f"ws{l}", [F, F if l < 2 else OUT_F], dt.bfloat16,
                         kind="ExternalInput") for l in range(3)]
    wn = [nc.dram_tensor(f"wn{l}", [F, F if l < 2 else OUT_F], dt.bfloat16,
                         kind="ExternalInput") for l in range(3)]
    bs = [nc.dram_tensor(f"b{l}", [F if l < 2 else OUT_F, 1], dt.float32,
                         kind="ExternalInput") for l in range(3)]
    outd = nc.dram_tensor("out", [NLOC, OUT_F], dt.float32, kind="ExternalOutput")

    htabs = [htab0,
             nc.dram_tensor("htab1", [NPAD, F], dt.bfloat16, addr_space="Shared"),
             nc.dram_tensor("htab2", [NPAD, F], dt.bfloat16, addr_space="Shared")]
    blks = [nc.dram_tensor(f"blk{l}", [NLOC, F], dt.bfloat16) for l in range(2)]

    with tile.TileContext(nc) as tc, ExitStack() as ctx:
        const = ctx.enter_context(tc.tile_pool(name="const", bufs=1))
        stpool = ctx.enter_context(tc.tile_pool(name="stage", bufs=1))
        msgp = ctx.enter_context(tc.tile_pool(name="msg", bufs=4))
        ohp = ctx.enter_context(tc.tile_pool(name="oh", bufs=2))
        sbw = ctx.enter_context(tc.tile_pool(name="work", bufs=3))
        psA = ctx.enter_context(tc.tile_pool(name="psA", bufs=2, space="PSUM"))
        psT = ctx.enter_context(tc.tile_pool(name="psT", bufs=1, space="PSUM"))
        psO = ctx.enter_context(tc.tile_pool(name="psO", bufs=2, space="PSUM"))

        def load(t, d):
            nc.sync.dma_start(t[:], d[:])
            return t

        idxlo_sb = load(const.tile([P, NTILES, NLO * 8], dt.int16, name="idxlo_sb"), idxlo)
        idxhi_sb = load(const.tile([P, NTILES, NHI * 8], dt.int16, name="idxhi_sb"), idxhi)
        dstlocf_sb = load(const.tile([P, NTILES, NCH], dt.float32, name="dstlocf_sb"), dstlocf)
        iotam_sb = load(const.tile([P, P], dt.float32, name="iotam_sb"), iotam)
        invdeg_sb = load(const.tile([P, NTILES], dt.float32, name="invdeg_sb"), invdeg)
        ident_sb = load(const.tile([P, P], dt.bfloat16, name="ident_sb"), ident)
        identf_sb = load(const.tile([OUT_F, OUT_F], dt.float32, name="identf_sb"), identf)
        ws_sb = [load(const.tile([F, F if l < 2 else OUT_F], dt.bfloat16, name=f"ws_sb{l}"), ws[l])
                 for l in range(3)]
        wn_sb = [load(const.tile([F, F if l < 2 else OUT_F], dt.bfloat16, name=f"wn_sb{l}"), wn[l])
                 for l in range(3)]
        bs_sb = [load(const.tile([F if l < 2 else OUT_F, 1], dt.float32, name=f"bs_sb{l}"), bs[l])
                 for l in range(3)]

        stageA = load(stpool.tile([P, NTILES, F], dt.bfloat16, name="stageA", tag="stA"), hself0)
        stageB = stpool.tile([P, NTILES, F], dt.bfloat16, tag="stB")
        outstage = stpool.tile([P, NTILES, OUT_F], dt.float32, tag="stO")

        stage_prev, stage_next = stageA, stageB
        for l in range(3):
            tab = htabs[l]
            OUTL = F if l < 2 else OUT_F
            for t in range(NTILES):
                msg = msgp.tile([P, NCH, F], dt.bfloat16, tag="msg")
                oh = ohp.tile([P, NCH, F], dt.bfloat16, tag="oh")
                GC = 23  # chunks per gather call (<=3071 idxs, scratch cap)
                for j in range(0, NLO, GC):
                    n = min(GC, NLO - j)
                    nc.gpsimd.dma_gather(
                        msg[:, j:j + n, :], tab[0:32768, :],
                        idxlo_sb[:, t, j * 8:(j + n) * 8],
                        num_idxs=n * P, num_idxs_reg=n * P, elem_size=F,
                        single_packet=False)
                for j in range(0, NHI, GC):
                    n = min(GC, NHI - j)
                    nc.gpsimd.dma_gather(
                        msg[:, NLO + j:NLO + j + n, :],
                        tab[HI_BASE:HI_BASE + 32768, :],
                        idxhi_sb[:, t, j * 8:(j + n) * 8],
                        num_idxs=n * P, num_idxs_reg=n * P, elem_size=F,
                        single_packet=False)
                for k in range(NCH):
                    nc.vector.tensor_scalar(
                        oh[:, k, :], iotam_sb[:],
                        dstlocf_sb[:, t, k:k + 1], None,
                        mybir.AluOpType.is_equal)

                agg = psA.tile([P, F], dt.float32, tag="agg")
                for k in range(NCH):
                    nc.tensor.matmul(agg[:], oh[:, k, :], msg[:, k, :],
                                     start=(k == 0), stop=(k == NCH - 1))
                hn = sbw.tile([P, F], dt.bfloat16, tag="hn")
                nc.vector.tensor_scalar_mul(hn[:], agg[:], invdeg_sb[:, t:t + 1])

                hsT_ps = psT.tile([F, P], dt.bfloat16, tag="hsT")
                nc.tensor.transpose(hsT_ps[:], stage_prev[:, t, :], ident_sb[:])
                hsT = sbw.tile([F, P], dt.bfloat16, tag="hsTs")
                nc.vector.tensor_copy(hsT[:], hsT_ps[:])

                hnT_ps = psT.tile([F, P], dt.bfloat16, tag="hnT")
                nc.tensor.transpose(hnT_ps[:], hn[:], ident_sb[:])
                hnT = sbw.tile([F, P], dt.bfloat16, tag="hnTs")
                nc.vector.tensor_copy(hnT[:], hnT_ps[:])

                outp = psO.tile([OUTL, P], dt.float32, tag="outp")
                nc.tensor.matmul(outp[:], ws_sb[l][:], hsT[:], start=True, stop=False)
                nc.tensor.matmul(outp[:], wn_sb[l][:], hnT[:], start=False, stop=True)

                if l < 2:
                    outT = sbw.tile([OUTL, P], dt.bfloat16, tag="outT")
                    nc.scalar.activation(outT[:], outp[:],
                                         mybir.ActivationFunctionType.Relu,
                                         bias=bs_sb[l][:], scale=1.0)
                    oT_ps = psT.tile([P, OUTL], dt.bfloat16, tag="oT")
                    nc.tensor.transpose(oT_ps[:], outT[:], ident_sb[:])
                    nc.vector.tensor_copy(stage_next[:, t, :], oT_ps[:])
                else:
                    outT = sbw.tile([OUTL, P], dt.float32, tag="outTf")
                    nc.vector.tensor_scalar_add(outT[:], outp[:], bs_sb[2][:])
                    oT_ps = psT.tile([P, OUTL], dt.float32, tag="oT2")
                    nc.tensor.transpose(oT_ps[:], outT[:], identf_sb[:])
                    nc.vector.tensor_copy(outstage[:, t, :], oT_ps[:])

            if l < 2:
                blk = blks[l]
                nc.sync.dma_start(
                    blk[:].rearrange("(t p) f -> p t f", p=P), stage_next[:])
                nc.gpsimd.collective_compute(
                    "AllGather", mybir.AluOpType.bypass,
                    replica_groups=[list(range(NCORES))],
                    ins=[blk[:]], outs=[htabs[l + 1][:]])
                stage_prev, stage_next = stage_next, stage_prev

        nc.sync.dma_start(outd[:].rearrange("(t p) f -> p t f", p=P), outstage[:])

    nc.compile()
    return nc


def kernel(features, src, dst, W0, b0, W1, b1, W2, b2):
    features = np.asarray(features, np.float32)
    src = np.asarray(src)
    dst = np.asarray(dst)

    per_core, NLO, NHI = _preprocess(src, dst)

    key = (NLO, NHI)
    if key not in _prog_cache:
        _prog_cache[key] = _build_program(NLO, NHI)
    nc = _prog_cache[key]

    bf = ml_dtypes.bfloat16
    feat_pad = np.zeros((NPAD, F), np.float32)
    feat_pad[:N_NODES] = features
    htab0 = feat_pad.astype(bf)
    ident = np.eye(P, dtype=bf)
    iotam = np.tile(np.arange(P, dtype=np.float32), (P, 1))
    Wl = [np.asarray(w, np.float32) for w in (W0, W1, W2)]
    bl = [np.asarray(b, np.float32).reshape(-1, 1) for b in (b0, b1, b2)]

    identf = np.eye(OUT_F, dtype=np.float32)
    common = dict(htab0=htab0, iotam=iotam, ident=ident, identf=identf)
    for l in range(3):
        common[f"ws{l}"] = Wl[l][:F].astype(bf)
        common[f"wn{l}"] = Wl[l][F:].astype(bf)
        common[f"b{l}"] = bl[l]

    in_maps = []
    for c in range(NCORES):
        m = dict(common)
        m.update(per_core[c])
        hs = feat_pad[c * NLOC:(c + 1) * NLOC].reshape(NTILES, P, F)
        m["hself0"] = np.ascontiguousarray(hs.transpose(1, 0, 2)).astype(bf)
        in_maps.append(m)

    from concourse.bass_utils import run_bass_kernel_spmd
    res = run_bass_kernel_spmd(nc, in_maps, core_ids=list(range(NCORES)))
    global last_result
    last_result = res
    out = np.concatenate([res.results[c]["out"] for c in range(NCORES)], axis=0)
    return np.ascontiguousarray(out[:N_NODES]).astype(np.float32)


last_result = None

